# revision 13
# baseline (speedup 1.0000x reference)
"""Trainium2 Bass kernel for nn_Graph_to_Featuremaps_savemem.

Math: the reference computes, per batch b,
    scores[b,p,n] = (res @ nfr)[b,p] + (x @ nfh)[b,n]
    attn = softmax_n(scores);  out[b,p,c] = (attn @ (x @ W))[b,p,c]
Softmax over n is shift-invariant, so the (res @ nfr)[b,p] term cancels:
    attn[b,p,:] = softmax(x[b] @ nfh)   (independent of p)
    out[b,c,h,w] = relu(((softmax(x[b]@nfh) @ x[b]) @ W)[c])   broadcast over (h,w)
res_feature never affects the output. The kernel is a tiny per-batch compute
(one 64-softmax + two small matmuls) followed by a 256 MB broadcast write --
pure HBM-write-bound, sharded batch-parallel over 8 cores (2 batches, 32 MB
written per core).

Device-side chain (inputs cast to bf16 on host, merged into one (128,385)
tensor so the load is a single DMA with 770 B/partition descriptors; X is
passed pre-transposed so there is no PE transpose; all matmuls are
single-pass bf16 with fp32 PSUM accumulation):
  s  = X @ nfh                 (128,1)  one matmul (lhsT = XT)
  e  = exp(s)                  (128,1)  bf16 out
  M  = X @ W                   (128,256) one matmul, copied to SBUF as bf16
  S_b = ONES[b-rows]^T @ e[b]  (128,1)  per-batch sum broadcast to all parts
  RC[:,b] = 1/S_b              (128,2)
  V[b,c] = M[b-rows,c]^T @ e[b] (128,1) per (batch, c-half)
  fill[b,c] = (0 max V) * RC[:,b]  broadcast to (128, 2048) on DVE
Output: per (batch, c-half) row-block, 8 x 1 MB DMAs (128 partitions x 8 KB
descriptors at 64 KB dest stride) alternating the two HWDGE rings (SP/ACT).
This shape measures at the per-packet floor (~315 ns / 8 KB descriptor,
~419 GB/s aggregate).  Full 128-partition DMAs are essential: partial
partition ranges collapse the descriptor spread onto a few SDMA engines
(measured 4x slowdown), and stride-0 source APs or contiguous-dest variants
measure ~14% slower per packet.
"""

import numpy as np

N_CORES = 8
B, NODES, HID, C, H, W = 16, 64, 128, 256, 128, 128
HWP = H * W  # 16384
B_LOC = B // N_CORES  # 2 batches per core
BN = B_LOC * NODES  # 128
FILL_F = 2048  # fill width for block 0 (8 KB descriptors, fast stream start)
BIG_F = 8192  # fill width for blocks 1-3 (32 KB descriptors)
NBLK = 4  # output row-blocks per core: (batch, c-half)

_NC_CACHE = {}


def build_nc():
    import concourse.bass as bass
    import concourse.bacc as bacc
    import concourse.mybir as mybir
    from concourse.tile import TileContext

    f32 = mybir.dt.float32
    bf16 = mybir.dt.bfloat16
    Alu = mybir.AluOpType
    Act = mybir.ActivationFunctionType

    nc = bacc.Bacc(None, target_bir_lowering=False, debug=False)
    # merged input: col 0 = nfh, cols 1:129 = X^T, cols 129:385 = W
    inp_d = nc.declare_dram_parameter("inp", [HID, 1 + BN + C], bf16, isOutput=False)
    out_d = nc.declare_dram_parameter("out", [B_LOC * C, HWP], f32, isOutput=True)

    with TileContext(nc) as tc:
        with (
            tc.tile_pool(name="singles", bufs=1) as singles,
            tc.tile_pool(name="psum", bufs=1, space="PSUM") as psum,
        ):
            # ---- input first so its DMA issues as early as possible ----
            INP = singles.tile([HID, 1 + BN + C], bf16, tag="INP")
            nc.sync.dma_start(out=INP[:], in_=inp_d[:])
            NFH = INP[:, 0:1]
            XT = INP[:, 1 : 1 + BN]
            Wt = INP[:, 1 + BN : 1 + BN + C]

            # ---- constants (no input deps) ----
            ONES128 = singles.tile([128, 128], bf16, tag="ONES128")
            nc.vector.memset(ONES128[:], 1.0)
            ZERO = singles.tile([128, FILL_F], f32, tag="ZERO")
            nc.vector.memset(ZERO[:], 0.0)

            # ---- s = X @ nfh ; e = exp(s) (bf16) ----
            s_ps = psum.tile([BN, 1], f32, tag="s")
            nc.tensor.matmul(s_ps[:], XT, NFH)
            e_col = singles.tile([BN, 1], bf16, tag="e_col")
            nc.scalar.activation(e_col[:], s_ps[:], Act.Exp)

            # ---- M = X @ W (independent of the e-chain) ----
            M_ps = psum.tile([BN, C], f32, tag="M")
            nc.tensor.matmul(M_ps[:], XT, Wt)
            M_sb = singles.tile([BN, C], bf16, tag="M_sb")
            nc.vector.tensor_copy(M_sb[:], M_ps[:])

            # ---- per-batch sums broadcast to all partitions; RC = 1/S ----
            RC = singles.tile([128, B_LOC], f32, tag="RC")
            S_ps = []
            for b in range(B_LOC):
                sl = slice(b * NODES, (b + 1) * NODES)
                sp = psum.tile([128, 1], f32, tag=f"S{b}")
                nc.tensor.matmul(sp[:], ONES128[sl, :], e_col[sl, :])
                S_ps.append(sp)
            for b in range(B_LOC):
                nc.vector.reciprocal(RC[:, b : b + 1], S_ps[b][:])

            # ---- V[b,c] = M[b-rows, c-half]^T @ e[b] : (128,1) each,
            # fill = (0 max V) * RC[:,b], then the output DMAs per block.
            # Block 0 uses a (128,2048) fill + 8 x 1 MB DMAs (8 KB descs) so
            # the stream starts as early as possible; blocks 1-3 use
            # (128,8192) fills (written in four proven-form chunks) + 2 x
            # 4 MB DMAs whose 32 KB descriptors amortize per-packet cost ----
            ndma = 0
            for j in range(NBLK):
                b, hf = divmod(j, C // 128)
                sl = slice(b * NODES, (b + 1) * NODES)
                vp = psum.tile([128, 1], f32, tag=f"V{j}")
                nc.tensor.matmul(
                    vp[:], M_sb[sl, hf * 128 : (hf + 1) * 128], e_col[sl, :]
                )
                bf = FILL_F if j == 0 else BIG_F
                fill = singles.tile([128, bf], f32, tag=f"fill{j}")
                for c in range(bf // FILL_F):
                    nc.vector.tensor_scalar(
                        fill[:, c * FILL_F : (c + 1) * FILL_F],
                        ZERO[:],
                        vp[:],
                        RC[:, b : b + 1],
                        op0=Alu.max,
                        op1=Alu.mult,
                    )
                r0 = j * 128
                for k in range(HWP // bf):
                    eng = nc.sync if ndma % 2 == 0 else nc.scalar
                    ndma += 1
                    eng.dma_start(
                        out=out_d[r0 : r0 + 128, k * bf : (k + 1) * bf],
                        in_=fill[:],
                    )
    nc.finalize()
    return nc


def get_nc():
    if "nc" not in _NC_CACHE:
        _NC_CACHE["nc"] = build_nc()
    return _NC_CACHE["nc"]


def make_in_maps(input, node_fea_for_hidden, weight):
    import ml_dtypes

    bf16 = ml_dtypes.bfloat16
    x = np.asarray(input, np.float32)[0]  # (B, NODES, HID)
    nfh = np.asarray(node_fea_for_hidden, np.float32).reshape(HID, 1)
    w = np.asarray(weight, np.float32)
    in_maps = []
    for i in range(N_CORES):
        xt = x[i * B_LOC : (i + 1) * B_LOC].reshape(BN, HID).T
        inp = np.concatenate([nfh, xt, w], axis=1).astype(bf16)
        in_maps.append({"inp": np.ascontiguousarray(inp)})
    return in_maps


def run_spmd(in_maps, trace=False, **kw):
    from concourse.bass_utils import run_bass_kernel_spmd

    return run_bass_kernel_spmd(get_nc(), in_maps, list(range(N_CORES)), trace=trace, **kw)


def kernel(input, res_feature, node_fea_for_res, node_fea_for_hidden, weight):
    res = run_spmd(make_in_maps(input, node_fea_for_hidden, weight)).results
    out = np.concatenate(
        [r["out"].reshape(B_LOC, C, H, W) for r in res], axis=0
    )
    return out


# revision 14
# speedup vs baseline: 1.0512x; 1.0512x over previous
"""Trainium2 Bass kernel for nn_Graph_to_Featuremaps_savemem.

Math: the reference computes, per batch b,
    scores[b,p,n] = (res @ nfr)[b,p] + (x @ nfh)[b,n]
    attn = softmax_n(scores);  out[b,p,c] = (attn @ (x @ W))[b,p,c]
Softmax over n is shift-invariant, so the (res @ nfr)[b,p] term cancels:
    attn[b,p,:] = softmax(x[b] @ nfh)   (independent of p)
    out[b,c,h,w] = relu(((softmax(x[b]@nfh) @ x[b]) @ W)[c])   broadcast over (h,w)
res_feature never affects the output. The kernel is a tiny per-batch compute
(one 64-softmax + two small matmuls) followed by a 256 MB broadcast write --
pure HBM-write-bound, sharded batch-parallel over 8 cores (2 batches, 32 MB
written per core).

Device-side chain (inputs cast to bf16 on host, merged into one (128,385)
tensor so the load is a single DMA with 770 B/partition descriptors; X is
passed pre-transposed so there is no PE transpose; all matmuls are
single-pass bf16 with fp32 PSUM accumulation):
  s  = X @ nfh                 (128,1)  one matmul (lhsT = XT)
  e  = exp(s)                  (128,1)  bf16 out
  M  = X @ W                   (128,256) one matmul, copied to SBUF as bf16
  S_b = ONES[b-rows]^T @ e[b]  (128,1)  per-batch sum broadcast to all parts
  RC[:,b] = 1/S_b              (128,2)
  V[b,c] = M[b-rows,c]^T @ e[b] (128,1) per (batch, c-half)
  fill[b,c] = (0 max V) * RC[:,b]  broadcast to (128, 2048) on DVE
Output: per (batch, c-half) row-block, full-128-partition DMAs alternating
the two HWDGE rings (SP/ACT).  Block 0 goes out as 8 x 1 MB DMAs (8 KB
descriptors, 315 ns/packet) so the stream starts as early as possible;
blocks 1-3 go out as 2 x 4 MB DMAs whose 32 KB descriptors amortize the
per-packet overhead (1219 ns/packet, ~427 GB/s aggregate measured).  Full
128-partition DMAs are essential: partial partition ranges collapse the
descriptor spread onto a few SDMA engines (measured 4x slowdown), and
stride-0 source APs measure ~14% slower per packet.
"""

import numpy as np

N_CORES = 8
B, NODES, HID, C, H, W = 16, 64, 128, 256, 128, 128
HWP = H * W  # 16384
B_LOC = B // N_CORES  # 2 batches per core
BN = B_LOC * NODES  # 128
FILL_F = 2048  # fill width for block 0 (8 KB descriptors, fast stream start)
BIG_F = 8192  # fill width for blocks 1-3 (32 KB descriptors)
NBLK = 4  # output row-blocks per core: (batch, c-half)

_NC_CACHE = {}


def build_nc():
    import concourse.bass as bass
    import concourse.bacc as bacc
    import concourse.mybir as mybir
    from concourse.tile import TileContext

    f32 = mybir.dt.float32
    bf16 = mybir.dt.bfloat16
    Alu = mybir.AluOpType
    Act = mybir.ActivationFunctionType

    nc = bacc.Bacc(None, target_bir_lowering=False, debug=False)
    # merged input: col 0 = nfh, cols 1:129 = X^T, cols 129:385 = W
    inp_d = nc.declare_dram_parameter("inp", [HID, 1 + BN + C], bf16, isOutput=False)
    out_d = nc.declare_dram_parameter("out", [B_LOC * C, HWP], f32, isOutput=True)

    with TileContext(nc) as tc:
        with (
            tc.tile_pool(name="singles", bufs=1) as singles,
            tc.tile_pool(name="psum", bufs=1, space="PSUM") as psum,
        ):
            # ---- input first so its DMA issues as early as possible ----
            INP = singles.tile([HID, 1 + BN + C], bf16, tag="INP")
            nc.sync.dma_start(out=INP[:], in_=inp_d[:])
            NFH = INP[:, 0:1]
            XT = INP[:, 1 : 1 + BN]
            Wt = INP[:, 1 + BN : 1 + BN + C]

            # ---- constants (no input deps) ----
            ONES128 = singles.tile([128, 128], bf16, tag="ONES128")
            nc.vector.memset(ONES128[:], 1.0)
            ZERO = singles.tile([128, FILL_F], f32, tag="ZERO")
            nc.vector.memset(ZERO[:], 0.0)

            # ---- s = X @ nfh ; e = exp(s) (bf16) ----
            s_ps = psum.tile([BN, 1], f32, tag="s")
            nc.tensor.matmul(s_ps[:], XT, NFH)
            e_col = singles.tile([BN, 1], bf16, tag="e_col")
            nc.scalar.activation(e_col[:], s_ps[:], Act.Exp)

            # ---- M = X @ W (independent of the e-chain) ----
            M_ps = psum.tile([BN, C], f32, tag="M")
            nc.tensor.matmul(M_ps[:], XT, Wt)
            M_sb = singles.tile([BN, C], bf16, tag="M_sb")
            nc.vector.tensor_copy(M_sb[:], M_ps[:])

            # ---- per-batch sums broadcast to all partitions; RC = 1/S ----
            RC = singles.tile([128, B_LOC], f32, tag="RC")
            S_ps = []
            for b in range(B_LOC):
                sl = slice(b * NODES, (b + 1) * NODES)
                sp = psum.tile([128, 1], f32, tag=f"S{b}")
                nc.tensor.matmul(sp[:], ONES128[sl, :], e_col[sl, :])
                S_ps.append(sp)
            for b in range(B_LOC):
                nc.vector.reciprocal(RC[:, b : b + 1], S_ps[b][:])

            # ---- V[b,c] = M[b-rows, c-half]^T @ e[b] : (128,1) each,
            # fill = (0 max V) * RC[:,b], then the output DMAs per block.
            # Block 0 uses a (128,2048) fill + 8 x 1 MB DMAs (8 KB descs) so
            # the stream starts as early as possible; blocks 1-3 use
            # (128,8192) fills (written in four proven-form chunks) + 2 x
            # 4 MB DMAs whose 32 KB descriptors amortize per-packet cost ----
            ndma = 0
            for j in range(NBLK):
                b, hf = divmod(j, C // 128)
                sl = slice(b * NODES, (b + 1) * NODES)
                vp = psum.tile([128, 1], f32, tag=f"V{j}")
                nc.tensor.matmul(
                    vp[:], M_sb[sl, hf * 128 : (hf + 1) * 128], e_col[sl, :]
                )
                bf = FILL_F if j == 0 else BIG_F
                fill = singles.tile([128, bf], f32, tag=f"fill{j}")
                for c in range(bf // FILL_F):
                    nc.vector.tensor_scalar(
                        fill[:, c * FILL_F : (c + 1) * FILL_F],
                        ZERO[:],
                        vp[:],
                        RC[:, b : b + 1],
                        op0=Alu.max,
                        op1=Alu.mult,
                    )
                r0 = j * 128
                for k in range(HWP // bf):
                    eng = nc.sync if ndma % 2 == 0 else nc.scalar
                    ndma += 1
                    eng.dma_start(
                        out=out_d[r0 : r0 + 128, k * bf : (k + 1) * bf],
                        in_=fill[:],
                    )
    nc.finalize()
    return nc


def get_nc():
    if "nc" not in _NC_CACHE:
        _NC_CACHE["nc"] = build_nc()
    return _NC_CACHE["nc"]


def make_in_maps(input, node_fea_for_hidden, weight):
    import ml_dtypes

    bf16 = ml_dtypes.bfloat16
    x = np.asarray(input, np.float32)[0]  # (B, NODES, HID)
    nfh = np.asarray(node_fea_for_hidden, np.float32).reshape(HID, 1)
    w = np.asarray(weight, np.float32)
    in_maps = []
    for i in range(N_CORES):
        xt = x[i * B_LOC : (i + 1) * B_LOC].reshape(BN, HID).T
        inp = np.concatenate([nfh, xt, w], axis=1).astype(bf16)
        in_maps.append({"inp": np.ascontiguousarray(inp)})
    return in_maps


def run_spmd(in_maps, trace=False, **kw):
    from concourse.bass_utils import run_bass_kernel_spmd

    return run_bass_kernel_spmd(get_nc(), in_maps, list(range(N_CORES)), trace=trace, **kw)


def kernel(input, res_feature, node_fea_for_res, node_fea_for_hidden, weight):
    res = run_spmd(make_in_maps(input, node_fea_for_hidden, weight)).results
    out = np.concatenate(
        [r["out"].reshape(B_LOC, C, H, W) for r in res], axis=0
    )
    return out


# revision 15
# speedup vs baseline: 1.1871x; 1.1293x over previous
"""Trainium2 Bass kernel for nn_Graph_to_Featuremaps_savemem.

Math: the reference computes, per batch b,
    scores[b,p,n] = (res @ nfr)[b,p] + (x @ nfh)[b,n]
    attn = softmax_n(scores);  out[b,p,c] = (attn @ (x @ W))[b,p,c]
Softmax over n is shift-invariant, so the (res @ nfr)[b,p] term cancels:
    attn[b,p,:] = softmax(x[b] @ nfh)   (independent of p)
    out[b,c,h,w] = relu(((softmax(x[b]@nfh) @ x[b]) @ W)[c])   broadcast over (h,w)
res_feature never affects the output. The kernel is a tiny per-batch compute
(one 64-softmax + two small matmuls) followed by a 256 MB broadcast write --
pure HBM-write-bound, sharded batch-parallel over 8 cores (2 batches, 32 MB
written per core).

Device-side chain (inputs cast to bf16 on host, merged into one (128,513)
tensor [nfh | X^T | X | W] so the load is a single DMA with 1 KB/partition
descriptors; all matmuls are single-pass bf16 with fp32 PSUM):
  s  = X @ nfh                 (128,1)  one matmul (lhsT = X^T)
  e  = exp(s)                  (128,1)  bf16 out
  U_b = X[b]^T @ e[b]          (128,1)  per batch; U needs only X and e, so it
                                        runs before W is even consumed
  S_b = ONES[b-rows]^T @ e[b]  (128,1)  per-batch sum broadcast to all parts
  RC[:,b] = 1/S_b              (128,2)
  V[b,c] = W[:,c-half]^T @ U_b (128,1)  per (batch, c-half)
  fill[b,c] = (0 max V) * RC[:,b]  broadcast on DVE (proven tensor_scalar
                                   form streaming a real ZERO tile)
Output: per (batch, c-half) row-block, full-128-partition DMAs alternating
the two HWDGE rings (SP/ACT).  Block 0's fill is written in two 1024-col
chunks and shipped as 2 x 0.5 MB + 7 x 1 MB DMAs so the stream starts as
early as possible; blocks 1-3 ship as 2 x 4 MB DMAs whose 32 KB descriptors
amortize per-packet overhead (1219 ns/packet, ~427 GB/s measured, vs 315 ns
per 8 KB packet = 419 GB/s).  Full 128-partition DMAs are essential: partial
partition ranges collapse the descriptor spread onto a few SDMA engines
(measured 4x slowdown), and stride-0 source APs are ~14% slower per packet.
"""

import numpy as np

N_CORES = 8
B, NODES, HID, C, H, W = 16, 64, 128, 256, 128, 128
HWP = H * W  # 16384
B_LOC = B // N_CORES  # 2 batches per core
BN = B_LOC * NODES  # 128
FILL_F = 2048  # fill width for block 0 (8 KB descriptors, fast stream start)
HEAD_F = 1024  # width of block 0's first two column chunks
BIG_F = 8192  # fill width for blocks 1-3 (32 KB descriptors)
NBLK = 4  # output row-blocks per core: (batch, c-half)

_NC_CACHE = {}


def build_nc():
    import concourse.bass as bass
    import concourse.bacc as bacc
    import concourse.mybir as mybir
    from concourse.tile import TileContext

    f32 = mybir.dt.float32
    bf16 = mybir.dt.bfloat16
    Alu = mybir.AluOpType
    Act = mybir.ActivationFunctionType

    nc = bacc.Bacc(None, target_bir_lowering=False, debug=False)
    # merged input: col 0 = nfh, cols 1:129 = X^T, 129:257 = X, 257:513 = W
    NC0, NC1, NC2, NC3 = 1, 1 + BN, 1 + BN + HID, 1 + BN + HID + C
    inp_d = nc.declare_dram_parameter("inp", [128, NC3], bf16, isOutput=False)
    out_d = nc.declare_dram_parameter("out", [B_LOC * C, HWP], f32, isOutput=True)

    with TileContext(nc) as tc:
        with (
            tc.tile_pool(name="singles", bufs=1) as singles,
            tc.tile_pool(name="psum", bufs=1, space="PSUM") as psum,
        ):
            # ---- input first so its DMA issues as early as possible ----
            INP = singles.tile([128, NC3], bf16, tag="INP")
            nc.sync.dma_start(out=INP[:], in_=inp_d[:])
            NFH = INP[:, 0:NC0]
            XT = INP[:, NC0:NC1]  # (hid, bn)
            Xr = INP[:, NC1:NC2]  # (bn, hid)
            Wt = INP[:, NC2:NC3]  # (hid, C)

            # ---- constants (no input deps) ----
            ONES128 = singles.tile([128, 128], bf16, tag="ONES128")
            nc.vector.memset(ONES128[:], 1.0)
            ZERO = singles.tile([128, FILL_F], f32, tag="ZERO")
            nc.vector.memset(ZERO[:], 0.0)

            # ---- s = X @ nfh ; e = exp(s) (bf16) ----
            s_ps = psum.tile([BN, 1], f32, tag="s")
            nc.tensor.matmul(s_ps[:], XT, NFH)
            e_col = singles.tile([BN, 1], bf16, tag="e_col")
            nc.scalar.activation(e_col[:], s_ps[:], Act.Exp)

            # ---- U_b = X[b]^T e[b]; S_b = ones^T e[b]; RC = 1/S ----
            RC = singles.tile([128, B_LOC], f32, tag="RC")
            U_sb = []
            for b in range(B_LOC):
                sl = slice(b * NODES, (b + 1) * NODES)
                up = psum.tile([HID, 1], f32, tag=f"U{b}")
                nc.tensor.matmul(up[:], Xr[sl, :], e_col[sl, :])
                us = singles.tile([HID, 1], bf16, tag=f"Us{b}")
                nc.vector.tensor_copy(us[:], up[:])
                U_sb.append(us)
                sp = psum.tile([128, 1], f32, tag=f"S{b}")
                nc.tensor.matmul(sp[:], ONES128[sl, :], e_col[sl, :])
                nc.vector.reciprocal(RC[:, b : b + 1], sp[:])

            # ---- V[b,c] = W[:,c-half]^T @ U_b, fills, output DMAs ----
            ndma = 0
            for j in range(NBLK):
                b, hf = divmod(j, C // 128)
                vp = psum.tile([128, 1], f32, tag=["s", "U0", "U1", "S0"][j])
                nc.tensor.matmul(
                    vp[:], Wt[:, hf * 128 : (hf + 1) * 128], U_sb[b][:]
                )
                bf = FILL_F if j == 0 else BIG_F
                fill = singles.tile([128, bf], f32, tag=f"fill{j}")
                chunk = HEAD_F if j == 0 else FILL_F
                for c in range(bf // chunk):
                    nc.vector.tensor_scalar(
                        fill[:, c * chunk : (c + 1) * chunk],
                        ZERO[:, 0:chunk],
                        vp[:],
                        RC[:, b : b + 1],
                        op0=Alu.max,
                        op1=Alu.mult,
                    )
                r0 = j * 128
                # block 0: first two DMAs are the 1024-col chunks, then 1 MB
                widths = (
                    [HEAD_F, HEAD_F] + [FILL_F] * ((HWP - FILL_F) // FILL_F)
                    if j == 0
                    else [BIG_F] * (HWP // BIG_F)
                )
                col = 0
                for wdt in widths:
                    eng = nc.sync if ndma % 2 == 0 else nc.scalar
                    ndma += 1
                    eng.dma_start(
                        out=out_d[r0 : r0 + 128, col : col + wdt],
                        in_=fill[:, col % bf : col % bf + wdt],
                    )
                    col += wdt
    nc.finalize()
    return nc


def get_nc():
    if "nc" not in _NC_CACHE:
        _NC_CACHE["nc"] = build_nc()
    return _NC_CACHE["nc"]


def make_in_maps(input, node_fea_for_hidden, weight):
    import ml_dtypes

    bf16 = ml_dtypes.bfloat16
    x = np.asarray(input, np.float32)[0]  # (B, NODES, HID)
    nfh = np.asarray(node_fea_for_hidden, np.float32).reshape(HID, 1)
    w = np.asarray(weight, np.float32)
    in_maps = []
    for i in range(N_CORES):
        xr = x[i * B_LOC : (i + 1) * B_LOC].reshape(BN, HID)
        inp = np.concatenate([nfh, xr.T, xr, w], axis=1).astype(bf16)
        in_maps.append({"inp": np.ascontiguousarray(inp)})
    return in_maps


def run_spmd(in_maps, trace=False, **kw):
    from concourse.bass_utils import run_bass_kernel_spmd

    return run_bass_kernel_spmd(get_nc(), in_maps, list(range(N_CORES)), trace=trace, **kw)


def kernel(input, res_feature, node_fea_for_res, node_fea_for_hidden, weight):
    res = run_spmd(make_in_maps(input, node_fea_for_hidden, weight)).results
    out = np.concatenate(
        [r["out"].reshape(B_LOC, C, H, W) for r in res], axis=0
    )
    return out
